# revision 36
# baseline (speedup 1.0000x reference)
"""Binarized 3x3 conv (stride 1, pad 1) + training-mode sync BatchNorm on 8 TRN2 cores.

Math: out = BN(conv2d(sign(x), sign(w)) + bias), BN over (N, H, W) per channel,
affine=False, training stats. The +bias cancels exactly inside BN (mean absorbs
it, var is shift-invariant), so it is not computed.

Distribution: data-parallel, 4 images per core. Per-channel batch statistics
are combined across cores with a tiny AllGather of (mean, E[x^2]) + local
reduction so the normalization uses exact global batch stats (sync-BN). The
channels are split into two halves, each with its own collective, so the first
half's normalize+store hides under the second half's conv.

Device algorithm (per core):
  - binarize weights/activations to fp8e4 (+-1 exact) with the ScalarE Sign
    LUT; weights are loaded contiguously (oc-major) and transposed to
    ic-on-partitions with 36 TensorE 128x128 transposes during the DMA head
  - conv as 9 shifted matmuls per output tile with fp8 DoubleRow perf mode
    (K=256 contracted per instruction). Images live in SBUF zero-padded at a
    57-element row pitch (the next row's left pad doubles as this row's right
    pad) so one contiguous 456-column moving operand covers 8 output rows
    (only 8/456 columns are discarded garbage).
  - per-tile channel stats via VectorE bn_stats/bn_aggr
  - per-half AllGather of [mean, E[x^2]] (1 KB), local sum, then
    (x - mean) * rsqrt(var + eps) via tensor_scalar / ACT Identity, DMA out.
"""

import numpy as np

import concourse.tile as tile
from concourse import bacc, bass_utils, masks, mybir

N_CORES = 8
IMGS = 4          # images per core
CCH = 256         # channels
H = W = 56
PW = 57           # padded row pitch: col 0 is the left zero-pad; the NEXT
                  # row's col 0 doubles as this row's right zero-pad
PROWS = 58        # row 0 and row 57 are the top/bottom zero-pad rows
PREG = 3312       # per-icb region: 58*57=3306 rounded up to a 16-multiple
                  # (DoubleRow k-tile stride must be 16B-aligned) + overrun slack
KK = 3
ROWS = 8          # output rows per PSUM tile
NT = H // ROWS    # 7 tiles per image
NMM = ROWS * PW   # 456 moving columns per matmul
BN_EPS = 1e-5

F32 = mybir.dt.float32
FP8 = mybir.dt.float8e4


def _emit(nc, tc, x_t, w_t, out_t, with_collective):
    x_ap = x_t.ap()      # [IMGS, 256, 56, 56]
    w_ap = w_t.ap()      # [256, 256, 3, 3]
    out_ap = out_t.ap()  # [IMGS, 256, 56, 56]

    from contextlib import ExitStack

    with ExitStack() as ctx:
        wstage = ctx.enter_context(tc.tile_pool(name="wstage", bufs=4))
        xstage = ctx.enter_context(tc.tile_pool(name="xstage", bufs=2))
        xpad_p = ctx.enter_context(tc.tile_pool(name="xpad", bufs=IMGS))
        wsb_p = ctx.enter_context(tc.tile_pool(name="wsb", bufs=2))
        osb_p = ctx.enter_context(tc.tile_pool(name="osb", bufs=2 * IMGS))
        stat_p = ctx.enter_context(tc.tile_pool(name="stats", bufs=2))
        small = ctx.enter_context(tc.tile_pool(name="small", bufs=1))
        psum_p = ctx.enter_context(tc.tile_pool(name="psum", bufs=6, space="PSUM"))
        dram = ctx.enter_context(tc.tile_pool(name="dram", bufs=2, space="DRAM"))

        # ---- pad buffers first (gpsimd is otherwise idle)
        xpads = []
        for img in range(IMGS):
            xp = xpad_p.tile([128, 2, PREG], FP8)  # [icp | icb, flat 57-pitch]
            nc.gpsimd.memset(
                xp[:].rearrange("p i r -> p (i r)").bitcast(mybir.dt.uint32), 0
            )
            xpads.append(xp)

        def load_img(img, row_chunks=1):
            rows = H // row_chunks
            for icb in range(2):
                for rc in range(row_chunks):
                    xs = xstage.tile([128, rows * W], F32, name="xs")
                    nc.sync.dma_start(
                        out=xs[:],
                        in_=x_ap[
                            img,
                            icb * 128 : (icb + 1) * 128,
                            rc * rows : (rc + 1) * rows,
                            :,
                        ].rearrange("c h w -> c (h w)"),
                    )
                    dst = xpads[img][:, icb, : PROWS * PW].rearrange(
                        "p (h w) -> p h w", w=PW
                    )[:, 1 + rc * rows : 1 + (rc + 1) * rows, 1 : W + 1]
                    nc.scalar.sign(
                        out=dst,
                        in_=xs[:].rearrange("p (h w) -> p h w", h=rows),
                    )

        # ---- weights. HBM layout [o, i, ky, kx] is oc-major, but the matmul
        # needs ic on partitions. Loading ic-on-partitions directly is a
        # 36B-granular DMA (~4x bandwidth waste), so instead: contiguous load
        # with oc on partitions, Sign to fp8, then 36 TensorE 128x128
        # transposes (PE is idle during the head anyway) + DVE copies into
        # the [icp | icb, k, oc] matmul layout.
        ident = small.tile([128, 128], FP8)
        masks.make_identity(nc, ident[:])
        wsbs = [
            wsb_p.tile([128, 2, KK * KK, 128], FP8, name="wsb") for _ in range(2)
        ]
        w_stages = []
        for ocb in range(2):  # ocb0 weights first so its matmuls start early
            ws = wstage.tile([128, 2304], F32, name="ws")  # [ocp | (ic k)]
            nc.scalar.dma_start(
                out=ws[:],
                in_=w_ap[ocb * 128 : (ocb + 1) * 128, :, :, :].rearrange(
                    "o i ky kx -> o (i ky kx)"
                ),
            )
            w_stages.append(ws)

        def transpose_weights(ocb):
            wt = wstage.tile([128, 2304], FP8, name="wt")  # sign, [ocp | (ic k)]
            nc.scalar.sign(out=wt[:], in_=w_stages[ocb][:])
            wt_v = wt[:].rearrange("p (i k) -> p i k", k=KK * KK)
            for icb in range(2):
                for k in range(KK * KK):
                    # fp8 PE-transpose writes PSUM with element step 2
                    tps = psum_p.tile([128, 256], FP8, name="tps", bufs=2)
                    tps_v = tps[:].rearrange("p (n two) -> p n two", two=2)[:, :, 0]
                    nc.tensor.transpose(
                        tps_v,
                        wt_v[:, icb * 128 : (icb + 1) * 128, k],
                        ident[:],
                    )
                    nc.vector.tensor_copy(out=wsbs[ocb][:, icb, k, :], in_=tps_v)

        # ocb0's weights first (sign_w0 ahead of the x signs on the ACT queue,
        # so the PE transposes start as soon as the w0 DMA lands), then image
        # 0, then ocb1's weights, then the remaining images.
        transpose_weights(0)
        load_img(0)
        transpose_weights(1)
        for img in range(1, IMGS):
            load_img(img)

        # ---- conv + per-ocb stats pipeline.
        # Each ocb half finishes conv, AllGathers its (mean, E[x^2]) stats,
        # then normalizes + stores while the other half is still convolving.
        eps_t = small.tile([128, 1], F32)
        nc.vector.memset(eps_t[:], BN_EPS)

        for ocb in range(2):
            stats = stat_p.tile([128, IMGS, NT, 6], F32, name="stats")
            osbs = []
            for img in range(IMGS):
                osb = osb_p.tile([128, H * W], F32)
                osbs.append(osb)
                osb_v = osb[:].rearrange("p (h w) -> p h w", h=H)
                xflat = xpads[img][:]  # [128, 2, PREG]
                for t in range(NT):
                    ps = psum_p.tile([128, NMM], F32)
                    ki = 0
                    for ky in range(KK):
                        for kx in range(KK):
                            s = (ROWS * t + ky) * PW + kx
                            nc.tensor.matmul(
                                ps[:],
                                lhsT=wsbs[ocb][:, :, ky * KK + kx, :],
                                rhs=xflat[:, :, s : s + NMM],
                                start=(ki == 0),
                                stop=(ki == 8),
                                perf_mode=mybir.MatmulPerfMode.DoubleRow,
                            )
                            ki += 1
                    psv = ps[:].rearrange("p (r w) -> p r w", r=ROWS)[:, :, 0:W]

                    nc.scalar.copy(out=osb_v[:, t * ROWS : (t + 1) * ROWS, :], in_=psv)
                    nc.vector.bn_stats(
                        out=stats[:, img, t, :],
                        in_=osb[:, t * ROWS * W : (t + 1) * ROWS * W],
                    )

            # local (mean, var) for this half of the channels
            mv = small.tile([128, 2], F32, name="mv")
            nc.vector.bn_aggr(
                out=mv[:], in_=stats[:].rearrange("p n t s -> p (n t s)")
            )
            send = small.tile([128, 2], F32, name="send")
            nc.vector.tensor_copy(out=send[:, 0:1], in_=mv[:, 0:1])
            # q = var + mean^2  (= local E[x^2])
            nc.vector.tensor_scalar(
                out=send[:, 1:2],
                in0=mv[:, 0:1],
                scalar1=mv[:, 0:1],
                scalar2=mv[:, 1:2],
                op0=mybir.AluOpType.mult,
                op1=mybir.AluOpType.add,
            )
            if with_collective:
                # AllGather has roughly half the latency floor of AllReduce
                # at this (tiny) size; sum the 8 shards locally on VectorE.
                cin = dram.tile([128, 2], F32, name="cin")
                cout = dram.tile([N_CORES * 128, 2], F32, name="cout")
                nc.gpsimd.dma_start(out=cin[:], in_=send[:])
                nc.gpsimd.collective_compute(
                    "AllGather",
                    mybir.AluOpType.bypass,
                    replica_groups=[list(range(N_CORES))],
                    ins=[cin.opt()],
                    outs=[cout.opt()],
                )
                recv_all = small.tile([128, N_CORES, 2], F32, name="recv_all")
                nc.gpsimd.dma_start(
                    out=recv_all[:],
                    in_=cout[:].rearrange("(r p) s -> p r s", r=N_CORES),
                )
                recv = small.tile([128, 2], F32, name="recv")
                nc.vector.tensor_reduce(
                    out=recv[:],
                    in_=recv_all[:].rearrange("p r s -> p s r"),
                    op=mybir.AluOpType.add,
                    axis=mybir.AxisListType.X,
                )
                inv_n = 1.0 / N_CORES
            else:
                recv = send
                inv_n = 1.0

            meang = small.tile([128, 1], F32, name="meang")
            varg = small.tile([128, 1], F32, name="varg")
            rstd = small.tile([128, 1], F32, name="rstd")
            nc.vector.tensor_scalar(
                out=meang[:],
                in0=recv[:, 0:1],
                scalar1=inv_n,
                scalar2=None,
                op0=mybir.AluOpType.mult,
            )
            # var = E[x^2] - mean^2
            nc.vector.tensor_scalar(
                out=varg[:],
                in0=meang[:],
                scalar1=meang[:],
                scalar2=None,
                op0=mybir.AluOpType.mult,
            )
            nc.vector.tensor_scalar(
                out=varg[:],
                in0=recv[:, 1:2],
                scalar1=inv_n,
                scalar2=varg[:],
                op0=mybir.AluOpType.mult,
                op1=mybir.AluOpType.subtract,
            )
            # rstd = 1 / sqrt(var + eps)
            nc.scalar.activation(
                out=rstd[:],
                in_=varg[:],
                func=mybir.ActivationFunctionType.Sqrt,
                bias=eps_t[:],
            )
            nc.vector.reciprocal(out=rstd[:], in_=rstd[:])
            # shift = -mean * rstd, for the ACT normalize path
            shift = small.tile([128, 1], F32, name="shift")
            nc.vector.tensor_scalar(
                out=shift[:],
                in0=meang[:],
                scalar1=rstd[:],
                scalar2=-1.0,
                op0=mybir.AluOpType.mult,
                op1=mybir.AluOpType.mult,
            )

            # normalize (split across DVE and ACT) + store (rotate DMA queues)
            out_dma_engines = [nc.sync, nc.gpsimd, nc.scalar, nc.sync]
            for img in range(IMGS):
                osb = osbs[img]
                # ACT helps only in the exposed tail (ocb1); during ocb0's
                # epilogue ACT is still feeding ocb1's conv with PSUM copies.
                if ocb == 0 or img % 2 == 0:
                    nc.vector.tensor_scalar(
                        out=osb[:],
                        in0=osb[:],
                        scalar1=meang[:],
                        scalar2=rstd[:],
                        op0=mybir.AluOpType.subtract,
                        op1=mybir.AluOpType.mult,
                    )
                else:
                    # out = Identity(in * rstd + (-mean * rstd))
                    nc.scalar.activation(
                        out=osb[:],
                        in_=osb[:],
                        func=mybir.ActivationFunctionType.Identity,
                        bias=shift[:],
                        scale=rstd[:],
                    )
                out_dma_engines[img % len(out_dma_engines)].dma_start(
                    out=out_ap[img, ocb * 128 : (ocb + 1) * 128, :, :].rearrange(
                        "c h w -> c (h w)"
                    ),
                    in_=osb[:],
                )


def build_nc(with_collective=True, num_devices=N_CORES):
    nc = bacc.Bacc(
        "TRN2", target_bir_lowering=False, debug=False, num_devices=num_devices
    )
    x_t = nc.dram_tensor("x", [IMGS, CCH, H, W], F32, kind="ExternalInput")
    w_t = nc.dram_tensor("w", [CCH, CCH, KK, KK], F32, kind="ExternalInput")
    out_t = nc.dram_tensor("out", [IMGS, CCH, H, W], F32, kind="ExternalOutput")
    with tile.TileContext(nc) as tc:
        _emit(nc, tc, x_t, w_t, out_t, with_collective)
    nc.compile()
    return nc


_NC_CACHE = {}


def _get_nc():
    if "nc" not in _NC_CACHE:
        _NC_CACHE["nc"] = build_nc()
    return _NC_CACHE["nc"]


def kernel(**inputs) -> np.ndarray:
    x = np.ascontiguousarray(np.asarray(inputs["x"], dtype=np.float32))
    w = np.ascontiguousarray(np.asarray(inputs["weight"], dtype=np.float32))
    assert x.shape == (N_CORES * IMGS, CCH, H, W), x.shape
    assert w.shape == (CCH, CCH, KK, KK), w.shape
    # bias is mathematically irrelevant: BN(out + b) == BN(out) for per-channel
    # bias under training-mode BN with affine=False.
    nc = _get_nc()
    in_maps = [
        {"x": np.ascontiguousarray(x[c * IMGS : (c + 1) * IMGS]), "w": w}
        for c in range(N_CORES)
    ]
    res = bass_utils.run_bass_kernel_spmd(
        nc, in_maps, core_ids=list(range(N_CORES)), trace=False
    )
    return np.concatenate(
        [res.results[c]["out"] for c in range(N_CORES)], axis=0
    ).astype(np.float32)


# revision 40
# speedup vs baseline: 1.0038x; 1.0038x over previous
"""Binarized 3x3 conv (stride 1, pad 1) + training-mode sync BatchNorm on 8 TRN2 cores.

Math: out = BN(conv2d(sign(x), sign(w)) + bias), BN over (N, H, W) per channel,
affine=False, training stats. The +bias cancels exactly inside BN (mean absorbs
it, var is shift-invariant), so it is not computed.

Distribution: data-parallel, 4 images per core. Per-channel batch statistics
are combined across cores with a tiny AllGather of (mean, E[x^2]) + local
reduction so the normalization uses exact global batch stats (sync-BN). The
channels are split into two halves, each with its own collective, so the first
half's normalize+store hides under the second half's conv.

Device algorithm (per core):
  - binarize weights/activations to fp8e4 (+-1 exact) with the ScalarE Sign
    LUT; weights are loaded contiguously (oc-major) and transposed to
    ic-on-partitions with 36 TensorE 128x128 transposes during the DMA head
  - conv as 9 shifted matmuls per output tile with fp8 DoubleRow perf mode
    (K=256 contracted per instruction). Images live in SBUF zero-padded at a
    57-element row pitch (the next row's left pad doubles as this row's right
    pad) so one contiguous 456-column moving operand covers 8 output rows
    (only 8/456 columns are discarded garbage).
  - per-tile channel stats via VectorE bn_stats/bn_aggr
  - per-half AllGather of [mean, E[x^2]] (1 KB), local sum, then
    (x - mean) * rsqrt(var + eps) via tensor_scalar / ACT Identity, DMA out.
"""

import numpy as np

import concourse.tile as tile
from concourse import bacc, bass_utils, masks, mybir

N_CORES = 8
IMGS = 4          # images per core
CCH = 256         # channels
H = W = 56
PW = 57           # padded row pitch: col 0 is the left zero-pad; the NEXT
                  # row's col 0 doubles as this row's right zero-pad
PROWS = 58        # row 0 and row 57 are the top/bottom zero-pad rows
PREG = 3312       # per-icb region: 58*57=3306 rounded up to a 16-multiple
                  # (DoubleRow k-tile stride must be 16B-aligned) + overrun slack
KK = 3
ROWS = 8          # output rows per PSUM tile
NT = H // ROWS    # 7 tiles per image
NMM = ROWS * PW   # 456 moving columns per matmul
BN_EPS = 1e-5

F32 = mybir.dt.float32
FP8 = mybir.dt.float8e4


def _emit(nc, tc, x_t, w_t, out_t, with_collective):
    x_ap = x_t.ap()      # [IMGS, 256, 56, 56]
    w_ap = w_t.ap()      # [256, 256, 3, 3]
    out_ap = out_t.ap()  # [IMGS, 256, 56, 56]

    from contextlib import ExitStack

    with ExitStack() as ctx:
        wstage = ctx.enter_context(tc.tile_pool(name="wstage", bufs=4))
        xstage = ctx.enter_context(tc.tile_pool(name="xstage", bufs=2))
        xpad_p = ctx.enter_context(tc.tile_pool(name="xpad", bufs=IMGS))
        wsb_p = ctx.enter_context(tc.tile_pool(name="wsb", bufs=2))
        osb_p = ctx.enter_context(tc.tile_pool(name="osb", bufs=2 * IMGS))
        stat_p = ctx.enter_context(tc.tile_pool(name="stats", bufs=2))
        small = ctx.enter_context(tc.tile_pool(name="small", bufs=1))
        psum_p = ctx.enter_context(tc.tile_pool(name="psum", bufs=6, space="PSUM"))
        dram = ctx.enter_context(tc.tile_pool(name="dram", bufs=2, space="DRAM"))

        # identity first so PE warm-up matmuls can start immediately
        ident = small.tile([128, 128], FP8)
        masks.make_identity(nc, ident[:])

        # ---- pad buffers first (gpsimd is otherwise idle)
        xpads = []
        for img in range(IMGS):
            xp = xpad_p.tile([128, 2, PREG], FP8)  # [icp | icb, flat 57-pitch]
            nc.gpsimd.memset(
                xp[:].rearrange("p i r -> p (i r)").bitcast(mybir.dt.uint32), 0
            )
            xpads.append(xp)

        def load_img(img, row_chunks=1):
            rows = H // row_chunks
            for icb in range(2):
                for rc in range(row_chunks):
                    xs = xstage.tile([128, rows * W], F32, name="xs")
                    nc.sync.dma_start(
                        out=xs[:],
                        in_=x_ap[
                            img,
                            icb * 128 : (icb + 1) * 128,
                            rc * rows : (rc + 1) * rows,
                            :,
                        ].rearrange("c h w -> c (h w)"),
                    )
                    dst = xpads[img][:, icb, : PROWS * PW].rearrange(
                        "p (h w) -> p h w", w=PW
                    )[:, 1 + rc * rows : 1 + (rc + 1) * rows, 1 : W + 1]
                    nc.scalar.sign(
                        out=dst,
                        in_=xs[:].rearrange("p (h w) -> p h w", h=rows),
                    )

        # ---- weights. HBM layout [o, i, ky, kx] is oc-major, but the matmul
        # needs ic on partitions. Loading ic-on-partitions directly is a
        # 36B-granular DMA (~4x bandwidth waste), so instead: contiguous load
        # with oc on partitions, Sign to fp8, then 36 TensorE 128x128
        # transposes (PE is idle during the head anyway) + DVE copies into
        # the [icp | icb, k, oc] matmul layout.
        wsbs = [
            wsb_p.tile([128, 2, KK * KK, 128], FP8, name="wsb") for _ in range(2)
        ]
        w_stages = []
        for ocb in range(2):  # ocb0 weights first so its matmuls start early
            ws = wstage.tile([128, 2304], F32, name="ws")  # [ocp | (ic k)]
            nc.scalar.dma_start(
                out=ws[:],
                in_=w_ap[ocb * 128 : (ocb + 1) * 128, :, :, :].rearrange(
                    "o i ky kx -> o (i ky kx)"
                ),
            )
            w_stages.append(ws)

        def transpose_weights(ocb):
            wt = wstage.tile([128, 2304], FP8, name="wt")  # sign, [ocp | (ic k)]
            nc.scalar.sign(out=wt[:], in_=w_stages[ocb][:])
            wt_v = wt[:].rearrange("p (i k) -> p i k", k=KK * KK)
            for icb in range(2):
                for k in range(KK * KK):
                    # fp8 PE-transpose writes PSUM with element step 2
                    tps = psum_p.tile([128, 256], FP8, name="tps", bufs=2)
                    tps_v = tps[:].rearrange("p (n two) -> p n two", two=2)[:, :, 0]
                    nc.tensor.transpose(
                        tps_v,
                        wt_v[:, icb * 128 : (icb + 1) * 128, k],
                        ident[:],
                    )
                    nc.vector.tensor_copy(out=wsbs[ocb][:, icb, k, :], in_=tps_v)

        def warm_pe(n_mms, lhsT=None):
            # Dummy matmuls keep the PE activity monitor (HAM) from holding
            # the array at its cold 1.2 GHz clock during the DMA head;
            # transposes don't count as PE-busy for HAM. Passing a lhsT that
            # depends on the weight transposes anchors a batch later in time
            # so the activity has no >3.4us holes before the first real MM.
            # Shares the 256B/partition "tps" slots -> stays within 8 banks.
            lhsT = ident[:, 0:64] if lhsT is None else lhsT
            m = lhsT.shape[-1]
            warm = psum_p.tile([m, 64], F32, name="warm", tag="tps", bufs=2)
            for _ in range(n_mms):
                nc.tensor.matmul(
                    warm[:], lhsT=lhsT, rhs=ident[:, 64:128],
                    start=True, stop=True,
                )

        # ocb0's weights first (sign_w0 ahead of the x signs on the ACT queue,
        # so the PE transposes start as soon as the w0 DMA lands), then image
        # 0, then ocb1's weights, then the remaining images.
        warm_pe(96)
        transpose_weights(0)
        warm_pe(32, lhsT=wsbs[0][:, 0, 0, 0:64])   # after first transpose
        warm_pe(32, lhsT=wsbs[0][:, 1, KK * KK - 1, 0:64])  # after last one
        load_img(0)
        transpose_weights(1)
        for img in range(1, IMGS):
            load_img(img)

        # ---- conv + per-ocb stats pipeline.
        # Each ocb half finishes conv, AllGathers its (mean, E[x^2]) stats,
        # then normalizes + stores while the other half is still convolving.
        eps_t = small.tile([128, 1], F32)
        nc.vector.memset(eps_t[:], BN_EPS)

        for ocb in range(2):
            stats = stat_p.tile([128, IMGS, NT, 6], F32, name="stats")
            osbs = []
            for img in range(IMGS):
                osb = osb_p.tile([128, H * W], F32)
                osbs.append(osb)
                osb_v = osb[:].rearrange("p (h w) -> p h w", h=H)
                xflat = xpads[img][:]  # [128, 2, PREG]
                for t in range(NT):
                    ps = psum_p.tile([128, NMM], F32)
                    ki = 0
                    for ky in range(KK):
                        for kx in range(KK):
                            s = (ROWS * t + ky) * PW + kx
                            nc.tensor.matmul(
                                ps[:],
                                lhsT=wsbs[ocb][:, :, ky * KK + kx, :],
                                rhs=xflat[:, :, s : s + NMM],
                                start=(ki == 0),
                                stop=(ki == 8),
                                perf_mode=mybir.MatmulPerfMode.DoubleRow,
                            )
                            ki += 1
                    psv = ps[:].rearrange("p (r w) -> p r w", r=ROWS)[:, :, 0:W]

                    nc.scalar.copy(out=osb_v[:, t * ROWS : (t + 1) * ROWS, :], in_=psv)
                    nc.vector.bn_stats(
                        out=stats[:, img, t, :],
                        in_=osb[:, t * ROWS * W : (t + 1) * ROWS * W],
                    )

            # local (mean, var) for this half of the channels
            mv = small.tile([128, 2], F32, name="mv")
            nc.vector.bn_aggr(
                out=mv[:], in_=stats[:].rearrange("p n t s -> p (n t s)")
            )
            send = small.tile([128, 2], F32, name="send")
            nc.vector.tensor_copy(out=send[:, 0:1], in_=mv[:, 0:1])
            # q = var + mean^2  (= local E[x^2])
            nc.vector.tensor_scalar(
                out=send[:, 1:2],
                in0=mv[:, 0:1],
                scalar1=mv[:, 0:1],
                scalar2=mv[:, 1:2],
                op0=mybir.AluOpType.mult,
                op1=mybir.AluOpType.add,
            )
            if with_collective:
                # AllGather has roughly half the latency floor of AllReduce
                # at this (tiny) size; sum the 8 shards locally on VectorE.
                cin = dram.tile([128, 2], F32, name="cin")
                cout = dram.tile([N_CORES * 128, 2], F32, name="cout")
                nc.gpsimd.dma_start(out=cin[:], in_=send[:])
                nc.gpsimd.collective_compute(
                    "AllGather",
                    mybir.AluOpType.bypass,
                    replica_groups=[list(range(N_CORES))],
                    ins=[cin.opt()],
                    outs=[cout.opt()],
                )
                recv_all = small.tile([128, N_CORES, 2], F32, name="recv_all")
                nc.gpsimd.dma_start(
                    out=recv_all[:],
                    in_=cout[:].rearrange("(r p) s -> p r s", r=N_CORES),
                )
                recv = small.tile([128, 2], F32, name="recv")
                nc.vector.tensor_reduce(
                    out=recv[:],
                    in_=recv_all[:].rearrange("p r s -> p s r"),
                    op=mybir.AluOpType.add,
                    axis=mybir.AxisListType.X,
                )
                inv_n = 1.0 / N_CORES
            else:
                recv = send
                inv_n = 1.0

            meang = small.tile([128, 1], F32, name="meang")
            varg = small.tile([128, 1], F32, name="varg")
            rstd = small.tile([128, 1], F32, name="rstd")
            nc.vector.tensor_scalar(
                out=meang[:],
                in0=recv[:, 0:1],
                scalar1=inv_n,
                scalar2=None,
                op0=mybir.AluOpType.mult,
            )
            # var = E[x^2] - mean^2
            nc.vector.tensor_scalar(
                out=varg[:],
                in0=meang[:],
                scalar1=meang[:],
                scalar2=None,
                op0=mybir.AluOpType.mult,
            )
            nc.vector.tensor_scalar(
                out=varg[:],
                in0=recv[:, 1:2],
                scalar1=inv_n,
                scalar2=varg[:],
                op0=mybir.AluOpType.mult,
                op1=mybir.AluOpType.subtract,
            )
            # rstd = 1 / sqrt(var + eps)
            nc.scalar.activation(
                out=rstd[:],
                in_=varg[:],
                func=mybir.ActivationFunctionType.Sqrt,
                bias=eps_t[:],
            )
            nc.vector.reciprocal(out=rstd[:], in_=rstd[:])
            # shift = -mean * rstd, for the ACT normalize path
            shift = small.tile([128, 1], F32, name="shift")
            nc.vector.tensor_scalar(
                out=shift[:],
                in0=meang[:],
                scalar1=rstd[:],
                scalar2=-1.0,
                op0=mybir.AluOpType.mult,
                op1=mybir.AluOpType.mult,
            )

            # normalize (split across DVE and ACT) + store (rotate DMA queues)
            out_dma_engines = [nc.sync, nc.gpsimd, nc.scalar, nc.sync]
            for img in range(IMGS):
                osb = osbs[img]
                # ACT helps only in the exposed tail (ocb1); during ocb0's
                # epilogue ACT is still feeding ocb1's conv with PSUM copies.
                if ocb == 0 or img % 2 == 0:
                    nc.vector.tensor_scalar(
                        out=osb[:],
                        in0=osb[:],
                        scalar1=meang[:],
                        scalar2=rstd[:],
                        op0=mybir.AluOpType.subtract,
                        op1=mybir.AluOpType.mult,
                    )
                else:
                    # out = Identity(in * rstd + (-mean * rstd))
                    nc.scalar.activation(
                        out=osb[:],
                        in_=osb[:],
                        func=mybir.ActivationFunctionType.Identity,
                        bias=shift[:],
                        scale=rstd[:],
                    )
                out_dma_engines[img % len(out_dma_engines)].dma_start(
                    out=out_ap[img, ocb * 128 : (ocb + 1) * 128, :, :].rearrange(
                        "c h w -> c (h w)"
                    ),
                    in_=osb[:],
                )


def build_nc(with_collective=True, num_devices=N_CORES):
    nc = bacc.Bacc(
        "TRN2", target_bir_lowering=False, debug=False, num_devices=num_devices
    )
    x_t = nc.dram_tensor("x", [IMGS, CCH, H, W], F32, kind="ExternalInput")
    w_t = nc.dram_tensor("w", [CCH, CCH, KK, KK], F32, kind="ExternalInput")
    out_t = nc.dram_tensor("out", [IMGS, CCH, H, W], F32, kind="ExternalOutput")
    with tile.TileContext(nc) as tc:
        _emit(nc, tc, x_t, w_t, out_t, with_collective)
    nc.compile()
    return nc


_NC_CACHE = {}


def _get_nc():
    if "nc" not in _NC_CACHE:
        _NC_CACHE["nc"] = build_nc()
    return _NC_CACHE["nc"]


def kernel(**inputs) -> np.ndarray:
    x = np.ascontiguousarray(np.asarray(inputs["x"], dtype=np.float32))
    w = np.ascontiguousarray(np.asarray(inputs["weight"], dtype=np.float32))
    assert x.shape == (N_CORES * IMGS, CCH, H, W), x.shape
    assert w.shape == (CCH, CCH, KK, KK), w.shape
    # bias is mathematically irrelevant: BN(out + b) == BN(out) for per-channel
    # bias under training-mode BN with affine=False.
    nc = _get_nc()
    in_maps = [
        {"x": np.ascontiguousarray(x[c * IMGS : (c + 1) * IMGS]), "w": w}
        for c in range(N_CORES)
    ]
    res = bass_utils.run_bass_kernel_spmd(
        nc, in_maps, core_ids=list(range(N_CORES)), trace=False
    )
    return np.concatenate(
        [res.results[c]["out"] for c in range(N_CORES)], axis=0
    ).astype(np.float32)


# revision 44
# speedup vs baseline: 1.0084x; 1.0046x over previous
"""Binarized 3x3 conv (stride 1, pad 1) + training-mode sync BatchNorm on 8 TRN2 cores.

Math: out = BN(conv2d(sign(x), sign(w)) + bias), BN over (N, H, W) per channel,
affine=False, training stats. The +bias cancels exactly inside BN (mean absorbs
it, var is shift-invariant), so it is not computed.

Distribution: data-parallel, 4 images per core. Per-channel batch statistics
are combined across cores with a tiny AllGather of (mean, E[x^2]) + local
reduction so the normalization uses exact global batch stats (sync-BN). The
channels are split into two halves, each with its own collective, so the first
half's normalize+store hides under the second half's conv.

Device algorithm (per core):
  - binarize weights/activations to fp8e4 (+-1 exact) with the ScalarE Sign
    LUT; weights are loaded contiguously (oc-major) and transposed to
    ic-on-partitions with 36 TensorE 128x128 transposes during the DMA head
  - conv as 9 shifted matmuls per output tile with fp8 DoubleRow perf mode
    (K=256 contracted per instruction). Images live in SBUF zero-padded at a
    57-element row pitch (the next row's left pad doubles as this row's right
    pad) so one contiguous 456-column moving operand covers 8 output rows
    (only 8/456 columns are discarded garbage).
  - per-tile channel stats via VectorE bn_stats/bn_aggr
  - per-half AllGather of [mean, E[x^2]] (1 KB), local sum, then
    (x - mean) * rsqrt(var + eps) via tensor_scalar / ACT Identity, DMA out.
"""

import numpy as np

import concourse.tile as tile
from concourse import bacc, bass_utils, masks, mybir

N_CORES = 8
IMGS = 4          # images per core
CCH = 256         # channels
H = W = 56
PW = 57           # padded row pitch: col 0 is the left zero-pad; the NEXT
                  # row's col 0 doubles as this row's right zero-pad
PROWS = 58        # row 0 and row 57 are the top/bottom zero-pad rows
PREG = 3312       # per-icb region: 58*57=3306 rounded up to a 16-multiple
                  # (DoubleRow k-tile stride must be 16B-aligned) + overrun slack
KK = 3
ROWS = 8          # output rows per PSUM tile
NT = H // ROWS    # 7 tiles per image
NMM = ROWS * PW   # 456 moving columns per matmul
BN_EPS = 1e-5

F32 = mybir.dt.float32
FP8 = mybir.dt.float8e4


def _emit(nc, tc, x_t, w_t, out_t, with_collective):
    x_ap = x_t.ap()      # [IMGS, 256, 56, 56]
    w_ap = w_t.ap()      # [256, 256, 3, 3]
    out_ap = out_t.ap()  # [IMGS, 256, 56, 56]

    from contextlib import ExitStack

    with ExitStack() as ctx:
        wstage = ctx.enter_context(tc.tile_pool(name="wstage", bufs=4))
        xstage = ctx.enter_context(tc.tile_pool(name="xstage", bufs=2))
        xpad_p = ctx.enter_context(tc.tile_pool(name="xpad", bufs=IMGS))
        wsb_p = ctx.enter_context(tc.tile_pool(name="wsb", bufs=2))
        osb_p = ctx.enter_context(tc.tile_pool(name="osb", bufs=2 * IMGS))
        stat_p = ctx.enter_context(tc.tile_pool(name="stats", bufs=2))
        small = ctx.enter_context(tc.tile_pool(name="small", bufs=1))
        psum_p = ctx.enter_context(tc.tile_pool(name="psum", bufs=6, space="PSUM"))
        dram = ctx.enter_context(tc.tile_pool(name="dram", bufs=2, space="DRAM"))

        # identity first so PE warm-up matmuls can start immediately
        ident = small.tile([128, 128], FP8)
        masks.make_identity(nc, ident[:])

        # ---- pad buffers first (gpsimd is otherwise idle)
        xpads = []
        for img in range(IMGS):
            xp = xpad_p.tile([128, 2, PREG], FP8)  # [icp | icb, flat 57-pitch]
            nc.gpsimd.memset(
                xp[:].rearrange("p i r -> p (i r)").bitcast(mybir.dt.uint32), 0
            )
            xpads.append(xp)

        def load_img(img, row_chunks=1):
            rows = H // row_chunks
            for icb in range(2):
                for rc in range(row_chunks):
                    xs = xstage.tile([128, rows * W], F32, name="xs")
                    nc.sync.dma_start(
                        out=xs[:],
                        in_=x_ap[
                            img,
                            icb * 128 : (icb + 1) * 128,
                            rc * rows : (rc + 1) * rows,
                            :,
                        ].rearrange("c h w -> c (h w)"),
                    )
                    dst = xpads[img][:, icb, : PROWS * PW].rearrange(
                        "p (h w) -> p h w", w=PW
                    )[:, 1 + rc * rows : 1 + (rc + 1) * rows, 1 : W + 1]
                    nc.scalar.sign(
                        out=dst,
                        in_=xs[:].rearrange("p (h w) -> p h w", h=rows),
                    )

        # ---- weights. HBM layout [o, i, ky, kx] is oc-major, but the matmul
        # needs ic on partitions. Loading ic-on-partitions directly is a
        # 36B-granular DMA (~4x bandwidth waste), so instead: contiguous load
        # with oc on partitions, Sign to fp8, then 36 TensorE 128x128
        # transposes (PE is idle during the head anyway) + DVE copies into
        # the [icp | icb, k, oc] matmul layout.
        wsbs = [
            wsb_p.tile([128, 2, KK * KK, 128], FP8, name="wsb") for _ in range(2)
        ]
        w_stages = {}

        def load_weights(ocb):
            ws = wstage.tile([128, 2304], F32, name="ws")  # [ocp | (ic k)]
            nc.scalar.dma_start(
                out=ws[:],
                in_=w_ap[ocb * 128 : (ocb + 1) * 128, :, :, :].rearrange(
                    "o i ky kx -> o (i ky kx)"
                ),
            )
            w_stages[ocb] = ws

        def transpose_weights(ocb):
            wt = wstage.tile([128, 2304], FP8, name="wt")  # sign, [ocp | (ic k)]
            nc.scalar.sign(out=wt[:], in_=w_stages[ocb][:])
            wt_v = wt[:].rearrange("p (i k) -> p i k", k=KK * KK)
            for icb in range(2):
                for k in range(KK * KK):
                    # fp8 PE-transpose writes PSUM with element step 2
                    tps = psum_p.tile([128, 256], FP8, name="tps", bufs=2)
                    tps_v = tps[:].rearrange("p (n two) -> p n two", two=2)[:, :, 0]
                    nc.tensor.transpose(
                        tps_v,
                        wt_v[:, icb * 128 : (icb + 1) * 128, k],
                        ident[:],
                    )
                    nc.vector.tensor_copy(out=wsbs[ocb][:, icb, k, :], in_=tps_v)

        def warm_pe(n_mms, lhsT=None):
            # Dummy matmuls keep the PE activity monitor (HAM) from holding
            # the array at its cold 1.2 GHz clock during the DMA head;
            # transposes don't count as PE-busy for HAM. Passing a lhsT that
            # depends on the weight transposes anchors a batch later in time
            # so the activity has no >3.4us holes before the first real MM.
            # Shares the 256B/partition "tps" slots -> stays within 8 banks.
            lhsT = ident[:, 0:64] if lhsT is None else lhsT
            m = lhsT.shape[-1]
            warm = psum_p.tile([m, 64], F32, name="warm", tag="tps", bufs=2)
            for _ in range(n_mms):
                nc.tensor.matmul(
                    warm[:], lhsT=lhsT, rhs=ident[:, 64:128],
                    start=True, stop=True,
                )

        # Head order: both weight chunks stream on the scalar-engine HWDGE
        # queue while x streams on sync; sign_w0 goes ahead of the x signs on
        # the ACT queue so the PE transposes start as soon as the w0 DMA
        # lands. Anchored warm-up batches keep HAM active through the head.
        load_weights(0)
        load_weights(1)
        warm_pe(96)
        transpose_weights(0)
        warm_pe(32, lhsT=wsbs[0][:, 0, 0, 0:64])   # after first transpose
        warm_pe(32, lhsT=wsbs[0][:, 1, KK * KK - 1, 0:64])  # after last one
        load_img(0)
        transpose_weights(1)
        for img in range(1, IMGS):
            load_img(img)

        # ---- conv + per-ocb stats pipeline.
        # Each ocb half finishes conv, AllGathers its (mean, E[x^2]) stats,
        # then normalizes + stores while the other half is still convolving.
        eps_t = small.tile([128, 1], F32)
        nc.vector.memset(eps_t[:], BN_EPS)

        for ocb in range(2):
            stats = stat_p.tile([128, IMGS, NT, 6], F32, name="stats")
            osbs = []
            for img in range(IMGS):
                osb = osb_p.tile([128, H * W], F32)
                osbs.append(osb)
                osb_v = osb[:].rearrange("p (h w) -> p h w", h=H)
                xflat = xpads[img][:]  # [128, 2, PREG]
                for t in range(NT):
                    ps = psum_p.tile([128, NMM], F32)
                    ki = 0
                    for ky in range(KK):
                        for kx in range(KK):
                            s = (ROWS * t + ky) * PW + kx
                            nc.tensor.matmul(
                                ps[:],
                                lhsT=wsbs[ocb][:, :, ky * KK + kx, :],
                                rhs=xflat[:, :, s : s + NMM],
                                start=(ki == 0),
                                stop=(ki == 8),
                                perf_mode=mybir.MatmulPerfMode.DoubleRow,
                            )
                            ki += 1
                    psv = ps[:].rearrange("p (r w) -> p r w", r=ROWS)[:, :, 0:W]

                    nc.scalar.copy(out=osb_v[:, t * ROWS : (t + 1) * ROWS, :], in_=psv)
                    nc.vector.bn_stats(
                        out=stats[:, img, t, :],
                        in_=osb[:, t * ROWS * W : (t + 1) * ROWS * W],
                    )

            # local (mean, var) for this half of the channels
            mv = small.tile([128, 2], F32, name="mv")
            nc.vector.bn_aggr(
                out=mv[:], in_=stats[:].rearrange("p n t s -> p (n t s)")
            )
            send = small.tile([128, 2], F32, name="send")
            nc.vector.tensor_copy(out=send[:, 0:1], in_=mv[:, 0:1])
            # q = var + mean^2  (= local E[x^2])
            nc.vector.tensor_scalar(
                out=send[:, 1:2],
                in0=mv[:, 0:1],
                scalar1=mv[:, 0:1],
                scalar2=mv[:, 1:2],
                op0=mybir.AluOpType.mult,
                op1=mybir.AluOpType.add,
            )
            if with_collective:
                # AllGather has roughly half the latency floor of AllReduce
                # at this (tiny) size; sum the 8 shards locally on VectorE.
                cin = dram.tile([128, 2], F32, name="cin")
                cout = dram.tile([N_CORES * 128, 2], F32, name="cout")
                nc.gpsimd.dma_start(out=cin[:], in_=send[:])
                nc.gpsimd.collective_compute(
                    "AllGather",
                    mybir.AluOpType.bypass,
                    replica_groups=[list(range(N_CORES))],
                    ins=[cin.opt()],
                    outs=[cout.opt()],
                )
                recv_all = small.tile([128, N_CORES, 2], F32, name="recv_all")
                nc.gpsimd.dma_start(
                    out=recv_all[:],
                    in_=cout[:].rearrange("(r p) s -> p r s", r=N_CORES),
                )
                recv = small.tile([128, 2], F32, name="recv")
                nc.vector.tensor_reduce(
                    out=recv[:],
                    in_=recv_all[:].rearrange("p r s -> p s r"),
                    op=mybir.AluOpType.add,
                    axis=mybir.AxisListType.X,
                )
                inv_n = 1.0 / N_CORES
            else:
                recv = send
                inv_n = 1.0

            meang = small.tile([128, 1], F32, name="meang")
            varg = small.tile([128, 1], F32, name="varg")
            rstd = small.tile([128, 1], F32, name="rstd")
            nc.vector.tensor_scalar(
                out=meang[:],
                in0=recv[:, 0:1],
                scalar1=inv_n,
                scalar2=None,
                op0=mybir.AluOpType.mult,
            )
            # var = E[x^2] - mean^2
            nc.vector.tensor_scalar(
                out=varg[:],
                in0=meang[:],
                scalar1=meang[:],
                scalar2=None,
                op0=mybir.AluOpType.mult,
            )
            nc.vector.tensor_scalar(
                out=varg[:],
                in0=recv[:, 1:2],
                scalar1=inv_n,
                scalar2=varg[:],
                op0=mybir.AluOpType.mult,
                op1=mybir.AluOpType.subtract,
            )
            # rstd = 1 / sqrt(var + eps)
            nc.scalar.activation(
                out=rstd[:],
                in_=varg[:],
                func=mybir.ActivationFunctionType.Sqrt,
                bias=eps_t[:],
            )
            nc.vector.reciprocal(out=rstd[:], in_=rstd[:])
            # shift = -mean * rstd, for the ACT normalize path
            shift = small.tile([128, 1], F32, name="shift")
            nc.vector.tensor_scalar(
                out=shift[:],
                in0=meang[:],
                scalar1=rstd[:],
                scalar2=-1.0,
                op0=mybir.AluOpType.mult,
                op1=mybir.AluOpType.mult,
            )

            # normalize (split across DVE and ACT) + store (rotate DMA
            # queues). Half-image granularity in the exposed half (ocb1) so
            # the first store starts as soon as half an image is normalized.
            out_dma_engines = [nc.sync, nc.gpsimd, nc.scalar]
            halves = 2 if ocb == 1 else 1
            hsz = H * W // halves
            qi = 0
            for img in range(IMGS):
                osb = osbs[img]
                for hf in range(halves):
                    sl = slice(hf * hsz, (hf + 1) * hsz)
                    # ACT helps only in the exposed tail (ocb1); during ocb0's
                    # epilogue ACT still feeds ocb1's conv with PSUM copies.
                    if ocb == 0 or img % 2 == 0:
                        nc.vector.tensor_scalar(
                            out=osb[:, sl],
                            in0=osb[:, sl],
                            scalar1=meang[:],
                            scalar2=rstd[:],
                            op0=mybir.AluOpType.subtract,
                            op1=mybir.AluOpType.mult,
                        )
                    else:
                        # out = Identity(in * rstd + (-mean * rstd))
                        nc.scalar.activation(
                            out=osb[:, sl],
                            in_=osb[:, sl],
                            func=mybir.ActivationFunctionType.Identity,
                            bias=shift[:],
                            scale=rstd[:],
                        )
                    nc_eng = out_dma_engines[qi % len(out_dma_engines)]
                    qi += 1
                    nc_eng.dma_start(
                        out=out_ap[
                            img, ocb * 128 : (ocb + 1) * 128, :, :
                        ].rearrange("c h w -> c (h w)")[:, sl],
                        in_=osb[:, sl],
                    )


def build_nc(with_collective=True, num_devices=N_CORES):
    nc = bacc.Bacc(
        "TRN2", target_bir_lowering=False, debug=False, num_devices=num_devices
    )
    x_t = nc.dram_tensor("x", [IMGS, CCH, H, W], F32, kind="ExternalInput")
    w_t = nc.dram_tensor("w", [CCH, CCH, KK, KK], F32, kind="ExternalInput")
    out_t = nc.dram_tensor("out", [IMGS, CCH, H, W], F32, kind="ExternalOutput")
    with tile.TileContext(nc) as tc:
        _emit(nc, tc, x_t, w_t, out_t, with_collective)
    nc.compile()
    return nc


_NC_CACHE = {}


def _get_nc():
    if "nc" not in _NC_CACHE:
        _NC_CACHE["nc"] = build_nc()
    return _NC_CACHE["nc"]


def kernel(**inputs) -> np.ndarray:
    x = np.ascontiguousarray(np.asarray(inputs["x"], dtype=np.float32))
    w = np.ascontiguousarray(np.asarray(inputs["weight"], dtype=np.float32))
    assert x.shape == (N_CORES * IMGS, CCH, H, W), x.shape
    assert w.shape == (CCH, CCH, KK, KK), w.shape
    # bias is mathematically irrelevant: BN(out + b) == BN(out) for per-channel
    # bias under training-mode BN with affine=False.
    nc = _get_nc()
    in_maps = [
        {"x": np.ascontiguousarray(x[c * IMGS : (c + 1) * IMGS]), "w": w}
        for c in range(N_CORES)
    ]
    res = bass_utils.run_bass_kernel_spmd(
        nc, in_maps, core_ids=list(range(N_CORES)), trace=False
    )
    return np.concatenate(
        [res.results[c]["out"] for c in range(N_CORES)], axis=0
    ).astype(np.float32)


# revision 45
# speedup vs baseline: 1.0380x; 1.0294x over previous
"""Binarized 3x3 conv (stride 1, pad 1) + training-mode sync BatchNorm on 8 TRN2 cores.

Math: out = BN(conv2d(sign(x), sign(w)) + bias), BN over (N, H, W) per channel,
affine=False, training stats. The +bias cancels exactly inside BN (mean absorbs
it, var is shift-invariant), so it is not computed.

Distribution: data-parallel, 4 images per core. Per-channel batch statistics
are combined across cores with a tiny AllGather of (mean, E[x^2]) + local
reduction so the normalization uses exact global batch stats (sync-BN). The
channels are split into two halves, each with its own collective, so the first
half's normalize+store hides under the second half's conv.

Device algorithm (per core):
  - binarize weights/activations to fp8e4 (+-1 exact) with the ScalarE Sign
    LUT; weights are loaded contiguously (oc-major) and transposed to
    ic-on-partitions with 36 TensorE 128x128 transposes during the DMA head
  - conv as 9 shifted matmuls per output tile with fp8 DoubleRow perf mode
    (K=256 contracted per instruction). Images live in SBUF zero-padded at a
    57-element row pitch (the next row's left pad doubles as this row's right
    pad) so one contiguous 456-column moving operand covers 8 output rows
    (only 8/456 columns are discarded garbage).
  - per-tile channel stats via VectorE bn_stats/bn_aggr
  - per-half AllGather of [mean, E[x^2]] (1 KB), local sum, then
    (x - mean) * rsqrt(var + eps) via tensor_scalar / ACT Identity, DMA out.
"""

import numpy as np

import concourse.tile as tile
from concourse import bacc, bass_utils, masks, mybir

N_CORES = 8
IMGS = 4          # images per core
CCH = 256         # channels
H = W = 56
PW = 57           # padded row pitch: col 0 is the left zero-pad; the NEXT
                  # row's col 0 doubles as this row's right zero-pad
PROWS = 58        # row 0 and row 57 are the top/bottom zero-pad rows
PREG = 3312       # per-icb region: 58*57=3306 rounded up to a 16-multiple
                  # (DoubleRow k-tile stride must be 16B-aligned) + overrun slack
KK = 3
ROWS = 8          # output rows per PSUM tile
NT = H // ROWS    # 7 tiles per image
NMM = ROWS * PW   # 456 moving columns per matmul
BN_EPS = 1e-5

F32 = mybir.dt.float32
FP8 = mybir.dt.float8e4


def _emit(nc, tc, x_t, w_t, out_t, with_collective):
    x_ap = x_t.ap()      # [IMGS, 256, 56, 56]
    w_ap = w_t.ap()      # [256, 256, 3, 3]
    out_ap = out_t.ap()  # [IMGS, 256, 56, 56]

    from contextlib import ExitStack

    with ExitStack() as ctx:
        wstage = ctx.enter_context(tc.tile_pool(name="wstage", bufs=2))
        xstage = ctx.enter_context(tc.tile_pool(name="xstage", bufs=3))
        xpad_p = ctx.enter_context(tc.tile_pool(name="xpad", bufs=IMGS))
        wsb_p = ctx.enter_context(tc.tile_pool(name="wsb", bufs=2))
        osb_p = ctx.enter_context(tc.tile_pool(name="osb", bufs=2 * IMGS))
        stat_p = ctx.enter_context(tc.tile_pool(name="stats", bufs=2))
        small = ctx.enter_context(tc.tile_pool(name="small", bufs=1))
        psum_p = ctx.enter_context(tc.tile_pool(name="psum", bufs=6, space="PSUM"))
        dram = ctx.enter_context(tc.tile_pool(name="dram", bufs=2, space="DRAM"))

        # identity first so PE warm-up matmuls can start immediately
        ident = small.tile([128, 128], FP8)
        masks.make_identity(nc, ident[:])

        # ---- pad buffers first (gpsimd is otherwise idle)
        xpads = []
        for img in range(IMGS):
            xp = xpad_p.tile([128, 2, PREG], FP8)  # [icp | icb, flat 57-pitch]
            nc.gpsimd.memset(
                xp[:].rearrange("p i r -> p (i r)").bitcast(mybir.dt.uint32), 0
            )
            xpads.append(xp)

        def load_img(img, row_chunks=1):
            rows = H // row_chunks
            for icb in range(2):
                for rc in range(row_chunks):
                    xs = xstage.tile([128, rows * W], F32, name="xs")
                    nc.sync.dma_start(
                        out=xs[:],
                        in_=x_ap[
                            img,
                            icb * 128 : (icb + 1) * 128,
                            rc * rows : (rc + 1) * rows,
                            :,
                        ].rearrange("c h w -> c (h w)"),
                    )
                    dst = xpads[img][:, icb, : PROWS * PW].rearrange(
                        "p (h w) -> p h w", w=PW
                    )[:, 1 + rc * rows : 1 + (rc + 1) * rows, 1 : W + 1]
                    nc.scalar.sign(
                        out=dst,
                        in_=xs[:].rearrange("p (h w) -> p h w", h=rows),
                    )

        # ---- weights. HBM layout [o, i, ky, kx] is oc-major, but the matmul
        # needs ic on partitions. Loading ic-on-partitions directly is a
        # 36B-granular DMA (~4x bandwidth waste), so instead: contiguous load
        # with oc on partitions, Sign to fp8, then 36 TensorE 128x128
        # transposes (PE is idle during the head anyway) + DVE copies into
        # the [icp | icb, k, oc] matmul layout.
        wsbs = [
            wsb_p.tile([128, 2, KK * KK, 128], FP8, name="wsb") for _ in range(2)
        ]
        w_stages = {}

        def load_weights(ocb):
            ws = wstage.tile([128, 2304], F32, name="ws")  # [ocp | (ic k)]
            nc.scalar.dma_start(
                out=ws[:],
                in_=w_ap[ocb * 128 : (ocb + 1) * 128, :, :, :].rearrange(
                    "o i ky kx -> o (i ky kx)"
                ),
            )
            w_stages[ocb] = ws

        def transpose_weights(ocb):
            wt = wstage.tile([128, 2304], FP8, name="wt")  # sign, [ocp | (ic k)]
            nc.scalar.sign(out=wt[:], in_=w_stages[ocb][:])
            wt_v = wt[:].rearrange("p (i k) -> p i k", k=KK * KK)
            for icb in range(2):
                for k in range(KK * KK):
                    # fp8 PE-transpose writes PSUM with element step 2
                    tps = psum_p.tile([128, 256], FP8, name="tps", bufs=2)
                    tps_v = tps[:].rearrange("p (n two) -> p n two", two=2)[:, :, 0]
                    nc.tensor.transpose(
                        tps_v,
                        wt_v[:, icb * 128 : (icb + 1) * 128, k],
                        ident[:],
                    )
                    nc.vector.tensor_copy(out=wsbs[ocb][:, icb, k, :], in_=tps_v)

        def warm_pe(n_mms, lhsT=None):
            # Dummy matmuls keep the PE activity monitor (HAM) from holding
            # the array at its cold 1.2 GHz clock during the DMA head;
            # transposes don't count as PE-busy for HAM. Passing a lhsT that
            # depends on the weight transposes anchors a batch later in time
            # so the activity has no >3.4us holes before the first real MM.
            # Shares the 256B/partition "tps" slots -> stays within 8 banks.
            lhsT = ident[:, 0:64] if lhsT is None else lhsT
            m = lhsT.shape[-1]
            warm = psum_p.tile([m, 64], F32, name="warm", tag="tps", bufs=2)
            for _ in range(n_mms):
                nc.tensor.matmul(
                    warm[:], lhsT=lhsT, rhs=ident[:, 64:128],
                    start=True, stop=True,
                )

        # Head order: both weight chunks stream on the scalar-engine HWDGE
        # queue while x streams on sync; sign_w0 goes ahead of the x signs on
        # the ACT queue so the PE transposes start as soon as the w0 DMA
        # lands. Anchored warm-up batches keep HAM active through the head.
        load_weights(0)
        load_weights(1)
        warm_pe(96)
        transpose_weights(0)
        warm_pe(32, lhsT=wsbs[0][:, 0, 0, 0:64])   # after first transpose
        warm_pe(32, lhsT=wsbs[0][:, 1, KK * KK - 1, 0:64])  # after last one
        load_img(0)
        transpose_weights(1)
        for img in range(1, IMGS):
            load_img(img)

        # ---- conv + per-ocb stats pipeline.
        # Each ocb half finishes conv, AllGathers its (mean, E[x^2]) stats,
        # then normalizes + stores while the other half is still convolving.
        eps_t = small.tile([128, 1], F32)
        nc.vector.memset(eps_t[:], BN_EPS)

        for ocb in range(2):
            stats = stat_p.tile([128, IMGS, NT, 6], F32, name="stats")
            osbs = []
            for img in range(IMGS):
                osb = osb_p.tile([128, H * W], F32)
                osbs.append(osb)
                osb_v = osb[:].rearrange("p (h w) -> p h w", h=H)
                xflat = xpads[img][:]  # [128, 2, PREG]
                for t in range(NT):
                    ps = psum_p.tile([128, NMM], F32)
                    ki = 0
                    for ky in range(KK):
                        for kx in range(KK):
                            s = (ROWS * t + ky) * PW + kx
                            nc.tensor.matmul(
                                ps[:],
                                lhsT=wsbs[ocb][:, :, ky * KK + kx, :],
                                rhs=xflat[:, :, s : s + NMM],
                                start=(ki == 0),
                                stop=(ki == 8),
                                perf_mode=mybir.MatmulPerfMode.DoubleRow,
                            )
                            ki += 1
                    psv = ps[:].rearrange("p (r w) -> p r w", r=ROWS)[:, :, 0:W]

                    nc.scalar.copy(out=osb_v[:, t * ROWS : (t + 1) * ROWS, :], in_=psv)
                    nc.vector.bn_stats(
                        out=stats[:, img, t, :],
                        in_=osb[:, t * ROWS * W : (t + 1) * ROWS * W],
                    )

            # local (mean, var) for this half of the channels
            mv = small.tile([128, 2], F32, name="mv")
            nc.vector.bn_aggr(
                out=mv[:], in_=stats[:].rearrange("p n t s -> p (n t s)")
            )
            send = small.tile([128, 2], F32, name="send")
            nc.vector.tensor_copy(out=send[:, 0:1], in_=mv[:, 0:1])
            # q = var + mean^2  (= local E[x^2])
            nc.vector.tensor_scalar(
                out=send[:, 1:2],
                in0=mv[:, 0:1],
                scalar1=mv[:, 0:1],
                scalar2=mv[:, 1:2],
                op0=mybir.AluOpType.mult,
                op1=mybir.AluOpType.add,
            )
            if with_collective:
                # AllGather has roughly half the latency floor of AllReduce
                # at this (tiny) size; sum the 8 shards locally on VectorE.
                cin = dram.tile([128, 2], F32, name="cin")
                cout = dram.tile([N_CORES * 128, 2], F32, name="cout")
                nc.gpsimd.dma_start(out=cin[:], in_=send[:])
                nc.gpsimd.collective_compute(
                    "AllGather",
                    mybir.AluOpType.bypass,
                    replica_groups=[list(range(N_CORES))],
                    ins=[cin.opt()],
                    outs=[cout.opt()],
                )
                recv_all = small.tile([128, N_CORES, 2], F32, name="recv_all")
                nc.gpsimd.dma_start(
                    out=recv_all[:],
                    in_=cout[:].rearrange("(r p) s -> p r s", r=N_CORES),
                )
                recv = small.tile([128, 2], F32, name="recv")
                nc.vector.tensor_reduce(
                    out=recv[:],
                    in_=recv_all[:].rearrange("p r s -> p s r"),
                    op=mybir.AluOpType.add,
                    axis=mybir.AxisListType.X,
                )
                inv_n = 1.0 / N_CORES
            else:
                recv = send
                inv_n = 1.0

            meang = small.tile([128, 1], F32, name="meang")
            varg = small.tile([128, 1], F32, name="varg")
            rstd = small.tile([128, 1], F32, name="rstd")
            nc.vector.tensor_scalar(
                out=meang[:],
                in0=recv[:, 0:1],
                scalar1=inv_n,
                scalar2=None,
                op0=mybir.AluOpType.mult,
            )
            # var = E[x^2] - mean^2
            nc.vector.tensor_scalar(
                out=varg[:],
                in0=meang[:],
                scalar1=meang[:],
                scalar2=None,
                op0=mybir.AluOpType.mult,
            )
            nc.vector.tensor_scalar(
                out=varg[:],
                in0=recv[:, 1:2],
                scalar1=inv_n,
                scalar2=varg[:],
                op0=mybir.AluOpType.mult,
                op1=mybir.AluOpType.subtract,
            )
            # rstd = 1 / sqrt(var + eps)
            nc.scalar.activation(
                out=rstd[:],
                in_=varg[:],
                func=mybir.ActivationFunctionType.Sqrt,
                bias=eps_t[:],
            )
            nc.vector.reciprocal(out=rstd[:], in_=rstd[:])
            # shift = -mean * rstd, for the ACT normalize path
            shift = small.tile([128, 1], F32, name="shift")
            nc.vector.tensor_scalar(
                out=shift[:],
                in0=meang[:],
                scalar1=rstd[:],
                scalar2=-1.0,
                op0=mybir.AluOpType.mult,
                op1=mybir.AluOpType.mult,
            )

            # normalize (split across DVE and ACT) + store (rotate DMA
            # queues). Half-image granularity in the exposed half (ocb1) so
            # the first store starts as soon as half an image is normalized.
            out_dma_engines = [nc.sync, nc.gpsimd, nc.scalar]
            halves = 2 if ocb == 1 else 1
            hsz = H * W // halves
            qi = 0
            for img in range(IMGS):
                osb = osbs[img]
                for hf in range(halves):
                    sl = slice(hf * hsz, (hf + 1) * hsz)
                    # ACT helps only in the exposed tail (ocb1); during ocb0's
                    # epilogue ACT still feeds ocb1's conv with PSUM copies.
                    if ocb == 0 or img % 2 == 0:
                        nc.vector.tensor_scalar(
                            out=osb[:, sl],
                            in0=osb[:, sl],
                            scalar1=meang[:],
                            scalar2=rstd[:],
                            op0=mybir.AluOpType.subtract,
                            op1=mybir.AluOpType.mult,
                        )
                    else:
                        # out = Identity(in * rstd + (-mean * rstd))
                        nc.scalar.activation(
                            out=osb[:, sl],
                            in_=osb[:, sl],
                            func=mybir.ActivationFunctionType.Identity,
                            bias=shift[:],
                            scale=rstd[:],
                        )
                    nc_eng = out_dma_engines[qi % len(out_dma_engines)]
                    qi += 1
                    nc_eng.dma_start(
                        out=out_ap[
                            img, ocb * 128 : (ocb + 1) * 128, :, :
                        ].rearrange("c h w -> c (h w)")[:, sl],
                        in_=osb[:, sl],
                    )


def build_nc(with_collective=True, num_devices=N_CORES):
    nc = bacc.Bacc(
        "TRN2", target_bir_lowering=False, debug=False, num_devices=num_devices
    )
    x_t = nc.dram_tensor("x", [IMGS, CCH, H, W], F32, kind="ExternalInput")
    w_t = nc.dram_tensor("w", [CCH, CCH, KK, KK], F32, kind="ExternalInput")
    out_t = nc.dram_tensor("out", [IMGS, CCH, H, W], F32, kind="ExternalOutput")
    with tile.TileContext(nc) as tc:
        _emit(nc, tc, x_t, w_t, out_t, with_collective)
    nc.compile()
    return nc


_NC_CACHE = {}


def _get_nc():
    if "nc" not in _NC_CACHE:
        _NC_CACHE["nc"] = build_nc()
    return _NC_CACHE["nc"]


def kernel(**inputs) -> np.ndarray:
    x = np.ascontiguousarray(np.asarray(inputs["x"], dtype=np.float32))
    w = np.ascontiguousarray(np.asarray(inputs["weight"], dtype=np.float32))
    assert x.shape == (N_CORES * IMGS, CCH, H, W), x.shape
    assert w.shape == (CCH, CCH, KK, KK), w.shape
    # bias is mathematically irrelevant: BN(out + b) == BN(out) for per-channel
    # bias under training-mode BN with affine=False.
    nc = _get_nc()
    in_maps = [
        {"x": np.ascontiguousarray(x[c * IMGS : (c + 1) * IMGS]), "w": w}
        for c in range(N_CORES)
    ]
    res = bass_utils.run_bass_kernel_spmd(
        nc, in_maps, core_ids=list(range(N_CORES)), trace=False
    )
    return np.concatenate(
        [res.results[c]["out"] for c in range(N_CORES)], axis=0
    ).astype(np.float32)
